# revision 1
# baseline (speedup 1.0000x reference)
"""Bass/Trainium2 kernel for nn_BasicBlock_73933567033945 (CDConv / gnn_message_passing).

Strategy: graph is a fixed +-8 sequence window inside each of 4 chains of
L=2048 nodes (verified against the src/dst inputs at runtime). Shard the
8192 nodes across 8 NeuronCores (1024 nodes each, half a chain) with an
8-node halo; all gathers become partition shifts materialized by PE
shift-matmuls, the per-edge kernel is a PE matmul against a block-diagonal
weight, the kern (x) h bilinear runs as 17 broadcast multiplies on DVE, and
the (offset, channel) contraction runs on the PE via PSUM-accumulated
transposes followed by Wk-chunk matmuls. Pure data parallel: no collectives.
"""
import numpy as np

B, L, C = 4, 2048, 128
N = B * L
W = 32
KC = 24
SEQ_L = 11
R = 12.0
WIN = 8
NEG_IN = 0.1
NEG_K = 0.2
NCORES = 8
NPC = N // NCORES          # 1024 nodes per core
TS = 112                   # output nodes per tile
NT = 10                    # tiles per core (9*112 + 16)
HR = 9 * TS + 128          # 1136 halo rows per core
K17 = 2 * WIN + 1          # 17 window offsets
S_HALF = SEQ_L // 2

_PROG = {}


def _sidx(k):
    return int(np.clip(k - WIN, -S_HALF, S_HALF)) + S_HALF


def _build_program():
    import concourse.tile as tile
    from concourse import mybir, bacc
    from concourse.bass_utils import run_bass_kernel_spmd  # noqa: F401 (import check)
    from contextlib import ExitStack

    f32 = mybir.dt.float32
    AF = mybir.ActivationFunctionType
    OP = mybir.AluOpType
    AX = mybir.AxisListType

    nc = bacc.Bacc("TRN2", target_bir_lowering=False, debug=False)

    def din(name, shape):
        return nc.dram_tensor(name, shape, f32, kind="ExternalInput").ap()

    x_slot = din("x_slot", [128, NT * C])
    xc_slot = din("xc_slot", [128, NT * C])
    po_slot = din("po_slot", [128, NT * 12])
    w_in = din("w_in", [C, W])
    ws_a = din("ws_a", [128, K17 * KC])
    ws_b = din("ws_b", [8, K17 * KC])
    wk_p = din("wk_p", [128, 6 * W])
    w_out = din("w_out", [W, C])
    ident = din("ident", [128, 128])
    shifts = din("shifts", [128, K17 * TS])
    w5r = din("w5r", [128, 3 * KC])
    b5r = din("b5r", [128, KC])
    maskd = din("maskd", [128, NT * K17])
    nclmp = din("nclmp", [128, NT])
    alph1 = din("alph1", [128, 1])
    alph2 = din("alph2", [128, 1])
    y = nc.dram_tensor("y", [NPC, C], f32, kind="ExternalOutput").ap()

    with tile.TileContext(nc) as tc, ExitStack() as ctx:
        pers = ctx.enter_context(tc.tile_pool(name="pers", bufs=1))

        def load(ap_in, shape, tag):
            t = pers.tile(shape, f32, tag=tag)
            nc.sync.dma_start(t[:], ap_in)
            return t

        x_all = load(x_slot, [128, NT * C], "x_all")
        xc_all = load(xc_slot, [128, NT * C], "xc_all")
        # phys: per slot j, 44 cols [h(32) | pos(3) | ori(9)]
        phys = pers.tile([128, NT * 44], f32, tag="phys")
        nc.sync.dma_start(
            phys[:].rearrange("p (j c) -> p j c", c=44)[:, :, 32:44],
            po_slot.rearrange("p (j c) -> p j c", c=12))
        w_in_sb = load(w_in, [C, W], "w_in")
        ws_a_sb = load(ws_a, [128, K17 * KC], "ws_a")
        ws_b_sb = load(ws_b, [8, K17 * KC], "ws_b")
        wk_sb = load(wk_p, [128, 6 * W], "wk")
        w_out_sb = load(w_out, [W, C], "w_out")
        id_sb = load(ident, [128, 128], "ident")
        sh_sb = load(shifts, [128, K17 * TS], "shifts")
        w5r_sb = load(w5r, [128, 3 * KC], "w5r")
        b5r_sb = load(b5r, [128, KC], "b5r")
        mask_sb = load(maskd, [128, NT * K17], "mask")
        ncl_sb = load(nclmp, [128, NT], "nclmp")
        a1_sb = load(alph1, [128, 1], "a1")
        a2_sb = load(alph2, [128, 1], "a2")
        bf16 = mybir.dt.bfloat16
        idb_sb = pers.tile([128, 128], bf16, tag="identb")
        nc.vector.tensor_copy(idb_sb[:], id_sb[:])


        # ---------------- Phase A: h = lrelu(lrelu(x) @ W_in) per slot -----
        with tc.tile_pool(name="pA", bufs=2) as pA, \
             tc.tile_pool(name="pAp", bufs=2, space="PSUM") as pAp:
            for j in range(NT):
                xl = pA.tile([128, C], f32, tag="xl")
                nc.scalar.activation(xl[:], x_all[:, j * C:(j + 1) * C],
                                     AF.Prelu, bias=0.0, scale=1.0,
                                     alpha=a1_sb[:, 0:1])
                xT_p = pAp.tile([128, 128], f32, tag="xT")
                nc.tensor.matmul(xT_p[:], xl[:], id_sb[:], is_transpose=True,
                                 start=True, stop=True)
                xT = pA.tile([128, 128], f32, tag="xTsb")
                nc.scalar.copy(xT[:], xT_p[:])
                hp = pAp.tile([128, W], f32, tag="hp")
                nc.tensor.matmul(hp[:], xT[:], w_in_sb[:], start=True, stop=True)
                nc.scalar.activation(phys[:, 44 * j:44 * j + W], hp[:],
                                     AF.Prelu, bias=0.0, scale=1.0,
                                     alpha=a1_sb[:, 0:1])

        # ---------------- Phase B: per output tile ------------------------
        wrk = ctx.enter_context(tc.tile_pool(name="wrk", bufs=2))
        tpool = ctx.enter_context(tc.tile_pool(name="tmp", bufs=4))
        ps = ctx.enter_context(tc.tile_pool(name="ps", bufs=1, space="PSUM"))
        ps2 = ctx.enter_context(tc.tile_pool(name="ps2", bufs=2, space="PSUM"))

        P = TS  # 112 active partitions
        for t in range(NT):
            # ---- neighborhood materialization via shift matmuls ----------
            # nb layout: k-block (44 cols = h|pos|ori) at col(k); k<=10 in
            # bank0 (44k), k>=11 in bank1 (512+44(k-11)) so no matmul output
            # crosses a PSUM bank boundary.
            def ncol(k):
                return 44 * k if k <= 10 else 512 + 44 * (k - 11)
            nb_p = ps.tile([P, 820], f32, tag="nb")
            for k in range(K17):
                nc.tensor.matmul(nb_p[:, ncol(k):ncol(k) + 44],
                                 sh_sb[:, TS * k:TS * (k + 1)],
                                 phys[:, 44 * t:44 * t + 44],
                                 start=(k in (0, 11)), stop=(k in (10, 16)),
                                 skip_group_check=True)
            nb = wrk.tile([P, 820], f32, tag="nb_sb")
            nc.scalar.copy(nb[:, 0:776], nb_p[:, 0:776])
            pos_c = nb[:, ncol(8) + 32:ncol(8) + 35]       # k=8 center
            ori_c = nb[:, ncol(8) + 35:ncol(8) + 44]

            def kview(k0, kn, off, width):
                # [(k: step 44, kn), (c: step 1, width)] view at block k0+off
                return nb[:, ncol(k0) + off:ncol(k0) + off + 44 * kn] \
                    .rearrange("p (k c) -> p k c", c=44)[:, :, 0:width]

            # ---- geometry -> delta_aug [P, (k,8)] ------------------------
            da = wrk.tile([P, K17 * 8], f32, tag="da")
            dav = da[:].rearrange("p (k d) -> p k d", d=8)
            D = wrk.tile([P, K17 * 3], f32, tag="D")
            Dv = D[:].rearrange("p (k a) -> p k a", a=3)
            nc.vector.tensor_sub(
                Dv[:, 0:11, :], kview(0, 11, 32, 3),
                pos_c.unsqueeze(1).broadcast_to([P, 11, 3]))
            nc.vector.tensor_sub(
                Dv[:, 11:17, :], kview(11, 6, 32, 3),
                pos_c.unsqueeze(1).broadcast_to([P, 6, 3]))
            sq = wrk.tile([P, K17 * 3], f32, tag="sq")
            nc.vector.tensor_mul(sq[:], D[:], D[:])
            d2 = wrk.tile([P, K17], f32, tag="d2")
            nc.vector.tensor_reduce(d2[:], sq[:].rearrange("p (k a) -> p k a", a=3),
                                    axis=AX.X, op=OP.add)
            # dist/R into delta slot 6 ; raw dist for direction
            nc.scalar.activation(dav[:, :, 6], d2[:], AF.Sqrt, bias=0.0,
                                 scale=1.0 / (R * R))
            dist = wrk.tile([P, K17], f32, tag="dist")
            nc.scalar.activation(dist[:], d2[:], AF.Sqrt, bias=0.0, scale=1.0)
            rec = wrk.tile([P, K17], f32, tag="rec")
            nc.vector.tensor_scalar_add(dist[:], dist[:], 1e-9)
            nc.vector.reciprocal(rec[:], dist[:])
            dirn = wrk.tile([P, K17 * 3], f32, tag="dirn")
            dirnv = dirn[:].rearrange("p (k a) -> p k a", a=3)
            nc.vector.tensor_mul(dirnv, Dv,
                                 rec[:].unsqueeze(-1).broadcast_to([P, K17, 3]))
            # local_a = sum_b Ri[a,b] * dirn[b]
            lm = wrk.tile([P, K17 * 9], f32, tag="lm")
            lmv = lm[:].rearrange("p (k a b) -> p k a b", a=3, b=3)
            nc.vector.tensor_mul(
                lmv,
                ori_c.rearrange("p (a b) -> p a b", b=3).unsqueeze(1)
                     .broadcast_to([P, K17, 3, 3]),
                dirn[:].rearrange("p (k b) -> p k b", b=3).unsqueeze(2)
                       .broadcast_to([P, K17, 3, 3]))
            nc.vector.tensor_reduce(dav[:, :, 0:3], lmv, axis=AX.X, op=OP.add)
            # ofeat_a = sum_b Ri[a,b] * Rj[a,b]
            ofm = wrk.tile([P, K17 * 9], f32, tag="ofm")
            ofmv = ofm[:].rearrange("p (k a b) -> p k a b", a=3, b=3)
            nc.vector.tensor_mul(
                ofmv[:, 0:11],
                kview(0, 11, 35, 9).rearrange("p k (a b) -> p k a b", b=3),
                ori_c.rearrange("p (a b) -> p a b", b=3).unsqueeze(1)
                     .broadcast_to([P, 11, 3, 3]))
            nc.vector.tensor_mul(
                ofmv[:, 11:17],
                kview(11, 6, 35, 9).rearrange("p k (a b) -> p k a b", b=3),
                ori_c.rearrange("p (a b) -> p a b", b=3).unsqueeze(1)
                     .broadcast_to([P, 6, 3, 3]))
            nc.vector.tensor_reduce(dav[:, :, 3:6], ofmv, axis=AX.X, op=OP.add)
            nc.vector.memset(dav[:, :, 7], 1.0)
            # chain-boundary mask (zeroes whole delta rows incl. bias)
            nc.vector.tensor_mul(
                dav, dav,
                mask_sb[0:P, K17 * t:K17 * (t + 1)].unsqueeze(-1)
                      .broadcast_to([P, K17, 8]))

            # ---- kern = lrelu(delta_aug @ WS, 0.2) -----------------------
            dT_p = ps.tile([128, 224], f32, tag="dT")
            nc.tensor.matmul(dT_p[:, 0:P], da[:, 0:128], id_sb[0:P, 0:P],
                             is_transpose=True, start=True, stop=False,
                             skip_group_check=True)
            nc.tensor.matmul(dT_p[0:8, P:P + P], da[:, 128:136], id_sb[0:P, 0:P],
                             is_transpose=True, start=False, stop=True,
                             skip_group_check=True)
            dT = wrk.tile([128, 224], f32, tag="dT_sb")
            nc.scalar.copy(dT[:], dT_p[:])
            pre_p = ps.tile([P, K17 * KC], f32, tag="pre")
            nc.tensor.matmul(pre_p[:], dT[:, 0:P], ws_a_sb[:], start=True,
                             stop=False, skip_group_check=True)
            nc.tensor.matmul(pre_p[:], dT[0:8, P:P + P], ws_b_sb[:], start=False,
                             stop=True, skip_group_check=True)
            kern = wrk.tile([P, K17 * KC], f32, tag="kern")
            nc.scalar.activation(kern[:], pre_p[:], AF.Prelu, bias=0.0,
                                 scale=1.0, alpha=a2_sb[0:P, 0:1])

            # ---- self-edge compensation into kern k=8 block --------------
            rn = wrk.tile([P, 3], f32, tag="rn")
            nc.vector.tensor_reduce(
                rn[:], ofm[:, 72:81].rearrange("p (a b) -> p a b", b=3),
                axis=AX.X, op=OP.add)
            pself = wrk.tile([P, KC], f32, tag="pself")
            nc.vector.scalar_tensor_tensor(pself[:], w5r_sb[0:P, 0:KC],
                                           rn[:, 0:1], b5r_sb[0:P, :],
                                           OP.mult, OP.add)
            nc.vector.scalar_tensor_tensor(pself[:], w5r_sb[0:P, KC:2 * KC],
                                           rn[:, 1:2], pself[:], OP.mult, OP.add)
            nc.vector.scalar_tensor_tensor(pself[:], w5r_sb[0:P, 2 * KC:3 * KC],
                                           rn[:, 2:3], pself[:], OP.mult, OP.add)
            kself = wrk.tile([P, KC], f32, tag="kself")
            nc.vector.scalar_tensor_tensor(kself[:], pself[:], NEG_K, pself[:],
                                           OP.mult, OP.max)
            nc.vector.tensor_scalar_mul(kself[:], kself[:], ncl_sb[0:P, t:t + 1])
            nc.gpsimd.tensor_add(kern[:, 8 * KC:9 * KC],
                                 kern[:, 8 * KC:9 * KC], kself[:])

            # ---- bilinear: tmp_k = kern_k (x) h_shift_k; PE transp-accum -
            aggT_p = ps.tile([128, 768], f32, tag="aggT")

            def tmp_mult(k, tag, eng):
                tm = tpool.tile([P, KC * W], bf16, tag=tag)
                eng.tensor_mul(
                    tm[:].rearrange("p (c w) -> p c w", w=W),
                    nb[:, ncol(k):ncol(k) + W].unsqueeze(1)
                      .broadcast_to([P, KC, W]),
                    kern[:, KC * k:KC * (k + 1)].unsqueeze(-1)
                        .broadcast_to([P, KC, W]))
                return tm

            def tmp_transp(k, tm):
                for b in range(6):
                    nc.tensor.matmul(
                        aggT_p[:, 128 * b:128 * b + P],
                        tm[:, 128 * b:128 * (b + 1)], idb_sb[0:P, 0:P],
                        start=(k == 0 and b in (0, 4)),
                        stop=(k == 16 and b in (3, 5)),
                        skip_group_check=True)

            for k in range(K17):
                tmp_transp(k, tmp_mult(k, "tmp", nc.vector))
            aggT = wrk.tile([128, 768], f32, tag="aggT_sb")
            nc.scalar.copy(aggT[:], aggT_p[:])

            # ---- conv = lrelu(agg @ Wk, 0.1) ; out = conv @ W_out + x ----
            co_p = ps2.tile([P, 240], f32, tag="co")
            for b in range(6):
                nc.tensor.matmul(co_p[0:W, 0:P], wk_sb[:, W * b:W * (b + 1)],
                                 aggT[:, 128 * b:128 * b + P],
                                 start=(b == 0), stop=(b == 5),
                                 skip_group_check=True)
            convL = wrk.tile([W, P], f32, tag="convL")
            nc.scalar.activation(convL[:], co_p[0:W, 0:P], AF.Prelu, bias=0.0,
                                 scale=1.0, alpha=a1_sb[0:W, 0:1])
            # start=True: zeroes this bank on partitions 0..111 (convT results
            # already consumed by the Prelu above; zeroing is per-partition-range)
            nc.tensor.matmul(co_p[:, P:P + 128], convL[:], w_out_sb[:],
                             start=True, stop=True, skip_group_check=True)
            out_sb = wrk.tile([P, C], f32, tag="out_sb")
            nc.vector.tensor_add(out_sb[:], co_p[:, P:P + 128],
                                 xc_all[0:P, C * t:C * t + C])
            cnt = min(TS, NPC - TS * t)
            nc.sync.dma_start(y[TS * t:TS * t + cnt, :], out_sb[0:cnt, :])

    nc.compile()
    return nc


def _expected_src_dst():
    i = np.arange(N)
    offs = np.arange(-WIN, WIN + 1)
    j = i[:, None] + offs[None, :]
    valid = ((j // L) == (i[:, None] // L)) & (j >= 0) & (j < N)
    j = np.where(valid, j, i[:, None])
    dst = np.repeat(i, offs.size).astype(np.int32)
    src = j.reshape(-1).astype(np.int32)
    return src, dst


def _host_inputs(x, pos, ori, W_in, Ws0, bs0, Wk, W_out):
    xf = np.ascontiguousarray(x.reshape(N, C), np.float32)
    pos = np.asarray(pos, np.float32)
    ori = np.asarray(ori, np.float32)

    # shared weights / constants
    WS = np.zeros((136, K17 * KC), np.float32)
    for k in range(K17):
        s = _sidx(k)
        WS[8 * k:8 * k + 7, KC * k:KC * (k + 1)] = Ws0[s]
        WS[8 * k + 7, KC * k:KC * (k + 1)] = bs0[s]
    wk_p = np.zeros((128, 6 * W), np.float32)
    for b in range(6):
        wk_p[:, W * b:W * (b + 1)] = Wk[128 * b:128 * (b + 1), :]
    shifts = np.zeros((128, K17 * TS), np.float32)
    for k in range(K17):
        for p in range(TS):
            shifts[p + k, TS * k + p] = 1.0
    w5r = np.tile(Ws0[5][3:6].reshape(1, 3 * KC), (128, 1)).astype(np.float32)
    b5r = np.tile(bs0[5].reshape(1, KC), (128, 1)).astype(np.float32)
    common = dict(
        w_in=np.ascontiguousarray(W_in, np.float32),
        ws_a=np.ascontiguousarray(WS[0:128]),
        ws_b=np.ascontiguousarray(WS[128:136]),
        wk_p=wk_p,
        w_out=np.ascontiguousarray(W_out, np.float32),
        ident=np.eye(128, dtype=np.float32),
        shifts=shifts,
        w5r=w5r, b5r=b5r,
        alph1=np.full((128, 1), NEG_IN, np.float32),
        alph2=np.full((128, 1), NEG_K, np.float32),
    )

    in_maps = []
    for ci in range(NCORES):
        s0 = ci * NPC
        g = s0 - WIN + np.arange(HR)
        ok = (g >= 0) & (g < N)
        gi = np.clip(g, 0, N - 1)
        x_pad = np.where(ok[:, None], xf[gi], 0.0).astype(np.float32)
        p_pad = np.where(ok[:, None], pos[gi], 0.0).astype(np.float32)
        o_pad = np.where(ok[:, None], ori[gi], 0.0).astype(np.float32)

        jj, pp = np.meshgrid(np.arange(NT), np.arange(128), indexing="ij")
        rows = (TS * jj + pp)            # [NT, 128] all < HR
        x_slot = x_pad[rows].transpose(1, 0, 2).reshape(128, NT * C)
        po_pad = np.concatenate([p_pad, o_pad], axis=1)  # [HR, 12]
        po_slot = po_pad[rows].transpose(1, 0, 2).reshape(128, NT * 12)
        rc = WIN + TS * jj + pp
        okc = rc < HR
        xc_slot = np.where(okc[:, :, None], x_pad[np.minimum(rc, HR - 1)], 0.0)
        xc_slot = xc_slot.transpose(1, 0, 2).reshape(128, NT * C).astype(np.float32)

        mask = np.zeros((128, NT, K17), np.float32)
        ncl = np.zeros((128, NT), np.float32)
        for t in range(NT):
            for p in range(min(TS, NPC - TS * t) if TS * t < NPC else 0):
                n = s0 + TS * t + p
                off = n % L
                v = ((off + np.arange(-WIN, WIN + 1)) >= 0) & \
                    ((off + np.arange(-WIN, WIN + 1)) < L)
                mask[p, t, :] = v.astype(np.float32)
                ncl[p, t] = K17 - v.sum()
        in_maps.append(dict(
            x_slot=x_slot, xc_slot=xc_slot, po_slot=po_slot,
            maskd=mask.reshape(128, NT * K17), nclmp=ncl, **common))
    return in_maps


def kernel(x, pos, seq, ori, W_in, Ws0, bs0, Wk, W_out, src, dst):
    exp_src, exp_dst = _expected_src_dst()
    assert np.array_equal(np.asarray(src), exp_src), "unexpected src graph"
    assert np.array_equal(np.asarray(dst), exp_dst), "unexpected dst graph"

    from concourse.bass_utils import run_bass_kernel_spmd

    if "nc" not in _PROG:
        _PROG["nc"] = _build_program()
    nc = _PROG["nc"]

    in_maps = _host_inputs(np.asarray(x), np.asarray(pos), np.asarray(ori),
                           np.asarray(W_in), np.asarray(Ws0), np.asarray(bs0),
                           np.asarray(Wk), np.asarray(W_out))
    res = run_bass_kernel_spmd(nc, in_maps, list(range(NCORES)))
    out = np.concatenate([res.results[i]["y"] for i in range(NCORES)], axis=0)
    return out.reshape(B, L, C).astype(np.float32)



# revision 26
# speedup vs baseline: 1.3363x; 1.3363x over previous
"""Bass/Trainium2 kernel for nn_BasicBlock_73933567033945 (CDConv / gnn_message_passing).

v2 "scatter-form" design. Graph is a fixed +-8 sequence window inside 4
chains of L=2048 nodes (asserted at runtime). 8192 nodes shard across 8
cores (1024 each), TS=112 output nodes per tile, 10 tiles, each backed
by a 128-row "slot" (core-halo rows 112t .. 112t+128).

Per tile, everything stays partition-aligned (the BIR verifier rejects
unaligned partition-offset operands):
 - geometry on the slot's dest rows (DVE/gpsimd/Act) -> delta_aug `da`
   [128, 18*32] bf16 (17 window offsets + 1 chain-boundary compensation
   slot folded in via lrelu positive homogeneity).
 - da is PE-transposed (5 chunks) to daT[(k,d), slot-node]; the 18
   per-k pre-matmuls read daT with FREE-AXIS shifted views, giving
   kernS_k[m, c] = kern[dest m-ksh] directly in source coordinates.
 - products tmS_k = kernS_k (bcast over w) * hX (h expanded over c) run
   on DVE scalar_tensor_tensor in 4x mode (all operands bf16 packed).
 - the per-k dest shift happens inside the PE transpose-accumulate:
   matmul(aggT += tmS_k^T @ S_k), S_k[m, j] = 1 iff j = m - ksh
   (host-built shifted identities).
 - aggT ((w,c)-major, 768 x 112) contracts with Wk (rows permuted to
   (w,c) order) into conv, then W_out and the identity add.
All matmuls are bf16 (fp32 streams 4x slower through the PE); fp32 only
for PSUM accumulation and pos handling.
"""
import numpy as np
import ml_dtypes

B, L, C = 4, 2048, 128
N = B * L
W = 32
KC = 24
SEQ_L = 11
R = 12.0
WIN = 8
NEG_IN = 0.1
NEG_K = 0.2
NCORES = 8
NPC = N // NCORES          # 1024 nodes per core
TS = 112                   # output nodes per tile
NT = 10                    # tiles per core (9*112 + 16)
HR = (NT - 1) * TS + 128   # 1136 padded rows per core
K17 = 17
K18 = 18                   # 17 offsets + compensation slot
S_HALF = SEQ_L // 2
DA = K18 * 32              # da cols (d padded 8->32 for aligned lhsT bases)

BF = ml_dtypes.bfloat16

_PROG = {}


def _sidx(k):
    return int(np.clip(k - WIN, -S_HALF, S_HALF)) + S_HALF


def _ksh(k):
    return k if k < K17 else WIN


def _build_program():
    import os
    import concourse.tile as tile
    from concourse import mybir, bacc
    from contextlib import ExitStack

    STAGE = int(os.environ.get("KSTAGE", "9"))

    f32 = mybir.dt.float32
    bf16 = mybir.dt.bfloat16
    AF = mybir.ActivationFunctionType
    OP = mybir.AluOpType
    AX = mybir.AxisListType

    nc = bacc.Bacc("TRN2", target_bir_lowering=False, debug=False)

    def din(name, shape, dt=f32):
        return nc.dram_tensor(name, shape, dt, kind="ExternalInput").ap()

    w_in = din("w_in", [128, W], bf16)
    ws_in = din("ws_sb", [128, K18 * KC], bf16)
    idb_in = din("idb", [128, 128], bf16)
    wk_in = din("wk_p", [128, 6 * W], bf16)
    w_out_in = din("w_out", [W, C], bf16)
    a1_in = din("alph1", [128, 1])
    nbp_in = din("nbp", [128, NT * K17 * 3])
    posc_in = din("posc", [128, NT * 3])
    nbo_in = din("nbo", [128, NT * K17 * 9], bf16)
    oric_in = din("oric", [128, NT * 9], bf16)
    maskb_in = din("maskb", [128, NT * K18], bf16)  # col 17 = ncl
    nclf_in = din("nclf", [128, NT])
    xT_in = din("xT_slot", [128, NT * C])
    s_in = din("s_mats", [128, K18 * TS], bf16)
    xc_in = din("xc_slot", [128, NT * C], bf16)
    y = nc.dram_tensor("y", [NPC, C], f32, kind="ExternalOutput").ap()

    with tile.TileContext(nc) as tc, ExitStack() as ctx:
        pers = ctx.enter_context(tc.tile_pool(name="pers", bufs=1))

        def load(ap_in, shape, tag, dt=f32):
            t = pers.tile(shape, dt, tag=tag)
            nc.sync.dma_start(t[:], ap_in)
            return t

        w_in_sb = load(w_in, [128, W], "w_in", bf16)
        ws_sb = load(ws_in, [128, K18 * KC], "ws", bf16)
        idb = load(idb_in, [128, 128], "idb", bf16)
        wk_sb = load(wk_in, [128, 6 * W], "wk", bf16)
        w_out_sb = load(w_out_in, [W, C], "w_out", bf16)
        a1_sb = load(a1_in, [128, 1], "a1")
        nbp_all = load(nbp_in, [128, NT * K17 * 3], "nbp")
        posc_all = load(posc_in, [128, NT * 3], "posc")
        nbo_all = load(nbo_in, [128, NT * K17 * 9], "nbo", bf16)
        oric_all = load(oric_in, [128, NT * 9], "oric", bf16)
        mask_all = load(maskb_in, [128, NT * K18], "maskb", bf16)
        ncl_all = load(nclf_in, [128, NT], "nclf")
        xT_all = load(xT_in, [128, NT * C], "xT")
        s_sb = load(s_in, [128, K18 * TS], "smats", bf16)
        xc_all = load(xc_in, [128, NT * C], "xc", bf16)

        # da / daT-sb ping-pong tiles; memset once so pad lanes stay 0
        da_t = [pers.tile([128, DA], bf16, tag=f"da{i}", name=f"da{i}")
                for i in range(2)]
        daTs_t = [pers.tile([128, 6 * 144], bf16, tag=f"daTs{i}",
                            name=f"daTs{i}") for i in range(2)]
        for i in range(2):
            nc.vector.memset(da_t[i][:], 0.0)
            nc.vector.memset(daTs_t[i][:], 0.0)

        wrk = ctx.enter_context(tc.tile_pool(name="wrk", bufs=2))
        tmp = ctx.enter_context(tc.tile_pool(name="tmp", bufs=4))
        # PSUM pools, declared in an order that keeps every accumulating
        # tile bank-aligned: daT 2 banks, pre+hp 1, agg 2x2, co 1 = 8.
        ps_daT = ctx.enter_context(
            tc.tile_pool(name="ps_daT", bufs=1, space="PSUM"))
        ps_pre = ctx.enter_context(
            tc.tile_pool(name="ps_pre", bufs=1, space="PSUM"))
        ps_agg = ctx.enter_context(
            tc.tile_pool(name="ps_agg", bufs=2, space="PSUM"))
        ps_co = ctx.enter_context(
            tc.tile_pool(name="ps_co", bufs=1, space="PSUM"))

        AGGC = (0, 112, 224, 336, 512, 624)   # aggT chunk cols (bank-safe)

        for t in range(NT):
            # ---------------- phase A for this slot: h, hX ----------------
            xlT = wrk.tile([128, C], bf16, tag="xlT")
            nc.vector.scalar_tensor_tensor(
                xlT[:], xT_all[:, C * t:C * (t + 1)], NEG_IN,
                xT_all[:, C * t:C * (t + 1)], OP.mult, OP.max)
            pre_p = ps_pre.tile([128, 512], f32, tag="pre")
            hp_p = pre_p[:, 480:512]
            nc.tensor.matmul(hp_p, xlT[:], w_in_sb[:], start=True,
                             stop=True, skip_group_check=True)
            h_sb = wrk.tile([128, W], bf16, tag="h")
            nc.scalar.activation(h_sb[:], hp_p, AF.Prelu, bias=0.0,
                                 scale=1.0, alpha=a1_sb[:, 0:1])
            hX = wrk.tile([128, W * KC], bf16, tag="hX")
            nc.vector.tensor_copy(
                hX[:].rearrange("p (w c) -> p w c", c=KC),
                h_sb[:].unsqueeze(-1).broadcast_to([128, W, KC]))

            # ---------------- geometry (dest rows of slot) ----------------
            da = da_t[t % 2]
            dav = da[:].rearrange("p (k d) -> p k d", d=32)
            nbp_t = nbp_all[:, 51 * t:51 * (t + 1)] \
                .rearrange("p (k a) -> p k a", a=3)
            posc_t = posc_all[:, 3 * t:3 * (t + 1)]
            nbo_t = nbo_all[:, 153 * t:153 * (t + 1)] \
                .rearrange("p (k e) -> p k e", e=9)
            oric_t = oric_all[:, 9 * t:9 * (t + 1)]
            mask_t = mask_all[:, K18 * t:K18 * (t + 1)]
            ncl_t = ncl_all[:, t:t + 1]

            D = wrk.tile([128, K17 * 3], bf16, tag="D")
            Dv = D[:].rearrange("p (k a) -> p k a", a=3)
            nc.vector.tensor_sub(Dv, nbp_t,
                                 posc_t.unsqueeze(1).broadcast_to([128, K17, 3]))
            sq = wrk.tile([128, K17 * 3], bf16, tag="sq")
            nc.gpsimd.tensor_mul(sq[:], D[:], D[:])
            d2 = wrk.tile([128, K17], f32, tag="d2")
            nc.vector.tensor_reduce(
                d2[:], sq[:].rearrange("p (k a) -> p k a", a=3),
                axis=AX.X, op=OP.add)
            nc.scalar.activation(dav[:, 0:K17, 6], d2[:], AF.Sqrt,
                                 bias=0.0, scale=1.0 / (R * R))
            dist = wrk.tile([128, K17], f32, tag="dist")
            nc.scalar.activation(dist[:], d2[:], AF.Sqrt, bias=0.0,
                                 scale=1.0)
            nc.vector.tensor_scalar_add(dist[:], dist[:], 1e-9)
            rs = wrk.tile([128, K17], f32, tag="rs")
            nc.vector.reciprocal(rs[:], dist[:])
            dirn = wrk.tile([128, K17 * 3], bf16, tag="dirn")
            nc.vector.scalar_tensor_tensor(
                dirn[:].rearrange("p (k a) -> p k a", a=3), Dv, 1.0,
                rs[:].unsqueeze(-1).broadcast_to([128, K17, 3]),
                OP.mult, OP.mult)
            lm = wrk.tile([128, K17 * 9], bf16, tag="lm")
            nc.vector.tensor_mul(
                lm[:].rearrange("p (k a b) -> p k a b", a=3, b=3),
                dirn[:].rearrange("p (k b) -> p k b", b=3).unsqueeze(2)
                       .broadcast_to([128, K17, 3, 3]),
                oric_t.rearrange("p (a b) -> p a b", b=3).unsqueeze(1)
                      .broadcast_to([128, K17, 3, 3]))
            lo = wrk.tile([128, K17 * 6], f32, tag="lo")
            lov = lo[:].rearrange("p (k d) -> p k d", d=6)
            nc.vector.tensor_reduce(
                lov[:, :, 0:3],
                lm[:].rearrange("p (k a b) -> p k a b", a=3, b=3),
                axis=AX.X, op=OP.add)
            om = wrk.tile([128, K17 * 9], bf16, tag="om")
            nc.vector.tensor_mul(
                om[:].rearrange("p (k a b) -> p k a b", a=3, b=3),
                nbo_t.rearrange("p k (a b) -> p k a b", b=3),
                oric_t.rearrange("p (a b) -> p a b", b=3).unsqueeze(1)
                      .broadcast_to([128, K17, 3, 3]))
            nc.vector.tensor_reduce(
                lov[:, :, 3:6],
                om[:].rearrange("p (k a b) -> p k a b", a=3, b=3),
                axis=AX.X, op=OP.add)
            nc.vector.tensor_copy(dav[:, 0:K17, 0:6], lov)
            nc.vector.tensor_scalar_mul(dav[:, 17, 3:6], lov[:, 8, 3:6],
                                        ncl_t)
            nc.vector.tensor_copy(dav[:, :, 7], mask_t)

            # ------------- daT = transpose(da), 6 chunks of 3 k-slots -----
            daTs = daTs_t[t % 2]
            if STAGE >= 2:
                daT_p = ps_daT.tile([128, 1024], f32, tag="daT")
                for g in range(6):
                    nc.tensor.matmul(daT_p[0:96, 128 * g:128 * (g + 1)],
                                     da[:, 96 * g:96 * (g + 1)], idb[:],
                                     start=True, stop=True,
                                     skip_group_check=True)
                nc.scalar.copy(
                    daTs[0:96].rearrange("p (g c) -> p g c", c=144)
                    [:, :, 8:136],
                    daT_p[0:96, 0:768].rearrange("p (g c) -> p g c", c=128))

            # ---------------- pre_k + prelu -> kernS ----------------------
            kernS = wrk.tile([128, K18 * KC], bf16, tag="kernS")
            if STAGE >= 3:
                for k in range(K18):
                    ksh = _ksh(k)
                    g = k // 3
                    c0 = 144 * g + 16 - ksh
                    # contract over the chunk's full 96 rows; ws_sb rows
                    # outside this k's 8 lanes are zero, so base stays 0
                    nc.tensor.matmul(pre_p[:, KC * k:KC * (k + 1)],
                                     daTs[0:96, c0:c0 + 128],
                                     ws_sb[0:96, KC * k:KC * (k + 1)],
                                     start=True, stop=True,
                                     skip_group_check=True)
                nc.scalar.activation(kernS[:], pre_p[:, 0:K18 * KC], AF.Prelu,
                                     bias=0.0, scale=1.0, alpha=a1_sb[:, 0:1])
            else:
                nc.vector.memset(kernS[:], 0.125)

            # ---------------- products + transpose-accumulate -------------
            aggTs = wrk.tile([128, 736], bf16, tag="aggTs")
            if STAGE < 1:
                nc.vector.memset(aggTs[:], 0.01)
            aggT_p = ps_agg.tile([128, 1024], f32, tag="aggT")
            for k in range(K18 if STAGE >= 1 else 0):
                tm = tmp.tile([128, W * KC], bf16, tag="tm")
                kv = kernS[:, KC * k:KC * (k + 1)].unsqueeze(1) \
                    .broadcast_to([128, W, KC])
                hv = hX[:].rearrange("p (w c) -> p w c", c=KC)
                tv = tm[:].rearrange("p (w c) -> p w c", c=KC)
                if k in (7, 12):
                    nc.gpsimd.tensor_mul(tv, kv, hv)
                else:
                    nc.vector.scalar_tensor_tensor(tv, kv, 1.0, hv,
                                                   OP.mult, OP.mult)
                for b in range(6):
                    nc.tensor.matmul(
                        aggT_p[:, AGGC[b]:AGGC[b] + TS],
                        tm[:, 128 * b:128 * (b + 1)],
                        s_sb[:, TS * k:TS * (k + 1)],
                        start=(k == 0 and b in (0, 4)),
                        stop=(k == K18 - 1 and b in (3, 5)),
                        skip_group_check=True)
            if STAGE >= 1:
                nc.scalar.copy(aggTs[:], aggT_p[:, 0:736])

            # ---------------- conv = lrelu(Wk @ aggT); out ----------------
            co_p = ps_co.tile([128, 512], f32, tag="co")
            for b in range(6):
                nc.tensor.matmul(co_p[0:W, 0:TS], wk_sb[:, W * b:W * (b + 1)],
                                 aggTs[:, AGGC[b]:AGGC[b] + TS],
                                 start=(b == 0), stop=(b == 5),
                                 skip_group_check=True)
            convL = tmp.tile([W, TS], bf16, tag="convL")
            nc.scalar.activation(convL[:], co_p[0:W, 0:TS], AF.Prelu,
                                 bias=0.0, scale=1.0, alpha=a1_sb[0:W, 0:1])
            nc.tensor.matmul(co_p[0:TS, 128:256], convL[:], w_out_sb[:],
                             start=True, stop=False, skip_group_check=True)
            nc.tensor.matmul(co_p[0:TS, 128:256], idb[0:TS, 0:TS],
                             xc_all[0:TS, C * t:C * (t + 1)],
                             start=False, stop=True, skip_group_check=True)
            out_sb = wrk.tile([TS, C], f32, tag="out_sb")
            nc.scalar.copy(out_sb[:], co_p[0:TS, 128:256])
            cnt = min(TS, NPC - TS * t)
            nc.sync.dma_start(y[TS * t:TS * t + cnt, :], out_sb[0:cnt, :])

    nc.compile()
    return nc


def _expected_src_dst():
    i = np.arange(N)
    offs = np.arange(-WIN, WIN + 1)
    j = i[:, None] + offs[None, :]
    valid = ((j // L) == (i[:, None] // L)) & (j >= 0) & (j < N)
    j = np.where(valid, j, i[:, None])
    dst = np.repeat(i, offs.size).astype(np.int32)
    src = j.reshape(-1).astype(np.int32)
    return src, dst


def _host_inputs(x, pos, ori, W_in, Ws0, bs0, Wk, W_out):
    xf = np.ascontiguousarray(x.reshape(N, C), np.float32)
    pos = np.asarray(pos, np.float32)
    ori = np.asarray(ori, np.float32)

    def bf(a):
        return np.asarray(a, BF)

    # Wk rows permuted to (w, c) order, in 6 chunks of 128 rows
    Wk_wc = np.empty_like(Wk)
    for c in range(KC):
        for w in range(W):
            Wk_wc[w * KC + c] = Wk[c * W + w]
    wk_p = np.zeros((128, 6 * W), np.float32)
    for b in range(6):
        wk_p[:, W * b:W * (b + 1)] = Wk_wc[128 * b:128 * (b + 1), :]

    # ws_sb: col-block k holds WS_k rows at partitions 32*(k%4)+d
    ws_sb = np.zeros((128, K18 * KC), np.float32)
    for k in range(K17):
        s = _sidx(k)
        r0 = 32 * (k % 3)
        ws_sb[r0:r0 + 7, KC * k:KC * (k + 1)] = Ws0[s]
        ws_sb[r0 + 7, KC * k:KC * (k + 1)] = bs0[s]
    r0 = 32 * (17 % 3)
    ws_sb[r0 + 3:r0 + 6, KC * 17:KC * 18] = Ws0[5][3:6]
    ws_sb[r0 + 7, KC * 17:KC * 18] = bs0[5]

    # shifted identities S_k[m, j] = 1 iff j = m - ksh
    s_mats = np.zeros((128, K18 * TS), np.float32)
    for k in range(K18):
        ksh = _ksh(k)
        for j in range(TS):
            m = j + ksh
            if 0 <= m < 128:
                s_mats[m, TS * k + j] = 1.0

    common = dict(
        w_in=bf(W_in),
        ws_sb=bf(ws_sb),
        s_mats=bf(s_mats),
        idb=bf(np.eye(128, dtype=np.float32)),
        wk_p=bf(wk_p),
        w_out=bf(W_out),
        alph1=np.full((128, 1), NEG_IN, np.float32),
    )

    offs = np.arange(-WIN, WIN + 1)
    in_maps = []
    for ci in range(NCORES):
        s0 = ci * NPC
        g = s0 - WIN + np.arange(HR)
        ok = (g >= 0) & (g < N)
        gi = np.clip(g, 0, N - 1)
        x_pad = np.where(ok[:, None], xf[gi], 0.0).astype(np.float32)
        pos_pad = np.where(ok[:, None], pos[gi], 0.0).astype(np.float32)
        ori_pad = np.where(ok[:, None], ori[gi], 0.0).astype(np.float32)

        jj, pp = np.meshgrid(np.arange(NT), np.arange(128), indexing="ij")
        rows = (TS * jj + pp)                      # [NT,128] pad-row index
        xT_slot = x_pad[rows].transpose(2, 0, 1).reshape(C, NT * 128)
        rc = WIN + TS * jj + pp
        okc = rc < HR
        xc_slot = np.where(okc[:, :, None], x_pad[np.minimum(rc, HR - 1)], 0.0)
        xc_slot = xc_slot.transpose(1, 0, 2).reshape(128, NT * C) \
            .astype(np.float32)

        gdest = s0 - WIN + rows                    # [NT,128] global dest node
        nb_g = gdest[:, :, None] + offs[None, None, :]
        valid = ((nb_g // L) == (gdest[:, :, None] // L)) \
            & (nb_g >= 0) & (nb_g < N)
        nb_gi = np.where(valid, np.clip(nb_g, 0, N - 1),
                         np.clip(gdest[:, :, None], 0, N - 1))
        nbp_a = np.where(valid[..., None], pos[nb_gi],
                         pos_pad[rows][:, :, None, :])      # [NT,128,17,3]
        nbo_a = np.where(valid[..., None], ori[nb_gi], 0.0)  # [NT,128,17,9]
        mask = valid.astype(np.float32)                      # [NT,128,17]
        ncl = (K17 - mask.sum(-1)).astype(np.float32)        # [NT,128]
        maskb = np.concatenate([mask, ncl[:, :, None]], axis=2)

        in_maps.append(dict(
            xT_slot=np.ascontiguousarray(xT_slot, np.float32),
            xc_slot=bf(xc_slot),
            nbp=nbp_a.transpose(1, 0, 2, 3).reshape(128, NT * K17 * 3)
                .astype(np.float32),
            posc=pos_pad[rows].transpose(1, 0, 2).reshape(128, NT * 3)
                .astype(np.float32),
            nbo=bf(nbo_a.transpose(1, 0, 2, 3).reshape(128, NT * K17 * 9)),
            oric=bf(ori_pad[rows].transpose(1, 0, 2).reshape(128, NT * 9)),
            maskb=bf(maskb.transpose(1, 0, 2).reshape(128, NT * K18)),
            nclf=np.ascontiguousarray(ncl.T, np.float32),
            **common))
    return in_maps


def kernel(x, pos, seq, ori, W_in, Ws0, bs0, Wk, W_out, src, dst):
    exp_src, exp_dst = _expected_src_dst()
    assert np.array_equal(np.asarray(src), exp_src), "unexpected src graph"
    assert np.array_equal(np.asarray(dst), exp_dst), "unexpected dst graph"

    from concourse.bass_utils import run_bass_kernel_spmd

    if "nc" not in _PROG:
        _PROG["nc"] = _build_program()
    nc = _PROG["nc"]

    in_maps = _host_inputs(np.asarray(x), np.asarray(pos), np.asarray(ori),
                           np.asarray(W_in), np.asarray(Ws0), np.asarray(bs0),
                           np.asarray(Wk), np.asarray(W_out))
    res = run_bass_kernel_spmd(nc, in_maps, list(range(NCORES)))
    out = np.concatenate([res.results[i]["y"] for i in range(NCORES)], axis=0)
    return out.reshape(B, L, C).astype(np.float32)


# revision 27
# speedup vs baseline: 1.4087x; 1.0542x over previous
"""Bass/Trainium2 kernel for nn_BasicBlock_73933567033945 (CDConv / gnn_message_passing).

v2 "scatter-form" design. Graph is a fixed +-8 sequence window inside 4
chains of L=2048 nodes (asserted at runtime). 8192 nodes shard across 8
cores (1024 each), TS=112 output nodes per tile, 10 tiles, each backed
by a 128-row "slot" (core-halo rows 112t .. 112t+128).

Per tile, everything stays partition-aligned (the BIR verifier rejects
unaligned partition-offset operands):
 - geometry on the slot's dest rows (DVE/gpsimd/Act) -> delta_aug `da`
   [128, 18*32] bf16 (17 window offsets + 1 chain-boundary compensation
   slot folded in via lrelu positive homogeneity).
 - da is PE-transposed (5 chunks) to daT[(k,d), slot-node]; the 18
   per-k pre-matmuls read daT with FREE-AXIS shifted views, giving
   kernS_k[m, c] = kern[dest m-ksh] directly in source coordinates.
 - products tmS_k = kernS_k (bcast over w) * hX (h expanded over c) run
   on DVE scalar_tensor_tensor in 4x mode (all operands bf16 packed).
 - the per-k dest shift happens inside the PE transpose-accumulate:
   matmul(aggT += tmS_k^T @ S_k), S_k[m, j] = 1 iff j = m - ksh
   (host-built shifted identities).
 - aggT ((w,c)-major, 768 x 112) contracts with Wk (rows permuted to
   (w,c) order) into conv, then W_out and the identity add.
All matmuls are bf16 (fp32 streams 4x slower through the PE); fp32 only
for PSUM accumulation and pos handling.
"""
import numpy as np
import ml_dtypes

B, L, C = 4, 2048, 128
N = B * L
W = 32
KC = 24
SEQ_L = 11
R = 12.0
WIN = 8
NEG_IN = 0.1
NEG_K = 0.2
NCORES = 8
NPC = N // NCORES          # 1024 nodes per core
TS = 112                   # output nodes per tile
NT = 10                    # tiles per core (9*112 + 16)
HR = (NT - 1) * TS + 128   # 1136 padded rows per core
K17 = 17
K18 = 18                   # 17 offsets + compensation slot
S_HALF = SEQ_L // 2
DA = K18 * 32              # da cols (d padded 8->32 for aligned lhsT bases)

BF = ml_dtypes.bfloat16

_PROG = {}


def _sidx(k):
    return int(np.clip(k - WIN, -S_HALF, S_HALF)) + S_HALF


def _ksh(k):
    return k if k < K17 else WIN


def _build_program():
    import os
    import concourse.tile as tile
    from concourse import mybir, bacc
    from contextlib import ExitStack

    STAGE = int(os.environ.get("KSTAGE", "9"))

    f32 = mybir.dt.float32
    bf16 = mybir.dt.bfloat16
    AF = mybir.ActivationFunctionType
    OP = mybir.AluOpType
    AX = mybir.AxisListType

    nc = bacc.Bacc("TRN2", target_bir_lowering=False, debug=False)

    def din(name, shape, dt=f32):
        return nc.dram_tensor(name, shape, dt, kind="ExternalInput").ap()

    w_in = din("w_in", [128, W], bf16)
    ws_in = din("ws_sb", [128, K18 * KC], bf16)
    idb_in = din("idb", [128, 128], bf16)
    wk_in = din("wk_p", [128, 6 * W], bf16)
    w_out_in = din("w_out", [W, C], bf16)
    a1_in = din("alph1", [128, 1])
    nbp_in = din("nbp", [128, NT * K17 * 3])
    posc_in = din("posc", [128, NT * 3])
    nbo_in = din("nbo", [128, NT * K17 * 9], bf16)
    oric_in = din("oric", [128, NT * 9], bf16)
    maskb_in = din("maskb", [128, NT * K18], bf16)  # col 17 = ncl
    nclf_in = din("nclf", [128, NT])
    xT_in = din("xT_slot", [128, NT * C])
    s_in = din("s_mats", [128, K18 * TS], bf16)
    xc_in = din("xc_slot", [128, NT * C], bf16)
    y = nc.dram_tensor("y", [NPC, C], f32, kind="ExternalOutput").ap()

    with tile.TileContext(nc) as tc, ExitStack() as ctx:
        pers = ctx.enter_context(tc.tile_pool(name="pers", bufs=1))

        def load(ap_in, shape, tag, dt=f32):
            t = pers.tile(shape, dt, tag=tag)
            nc.sync.dma_start(t[:], ap_in)
            return t

        w_in_sb = load(w_in, [128, W], "w_in", bf16)
        ws_sb = load(ws_in, [128, K18 * KC], "ws", bf16)
        idb = load(idb_in, [128, 128], "idb", bf16)
        wk_sb = load(wk_in, [128, 6 * W], "wk", bf16)
        w_out_sb = load(w_out_in, [W, C], "w_out", bf16)
        a1_sb = load(a1_in, [128, 1], "a1")
        nbp_all = load(nbp_in, [128, NT * K17 * 3], "nbp")
        posc_all = load(posc_in, [128, NT * 3], "posc")
        nbo_all = load(nbo_in, [128, NT * K17 * 9], "nbo", bf16)
        oric_all = load(oric_in, [128, NT * 9], "oric", bf16)
        mask_all = load(maskb_in, [128, NT * K18], "maskb", bf16)
        ncl_all = load(nclf_in, [128, NT], "nclf")
        xT_all = load(xT_in, [128, NT * C], "xT")
        s_sb = load(s_in, [128, K18 * TS], "smats", bf16)
        xc_all = load(xc_in, [128, NT * C], "xc", bf16)

        # da / daT-sb ping-pong tiles; memset once so pad lanes stay 0
        da_t = [pers.tile([128, DA], bf16, tag=f"da{i}", name=f"da{i}")
                for i in range(2)]
        daTs_t = [pers.tile([128, 6 * 144], bf16, tag=f"daTs{i}",
                            name=f"daTs{i}") for i in range(2)]
        for i in range(2):
            nc.vector.memset(da_t[i][:], 0.0)
            nc.vector.memset(daTs_t[i][:], 0.0)

        wrk = ctx.enter_context(tc.tile_pool(name="wrk", bufs=2))
        tmp = ctx.enter_context(tc.tile_pool(name="tmp", bufs=4))
        # PSUM pools, declared in an order that keeps every accumulating
        # tile bank-aligned: daT 2 banks, pre+hp 1, agg 2x2, co 1 = 8.
        ps_daT = ctx.enter_context(
            tc.tile_pool(name="ps_daT", bufs=1, space="PSUM"))
        ps_pre = ctx.enter_context(
            tc.tile_pool(name="ps_pre", bufs=1, space="PSUM"))
        ps_agg = ctx.enter_context(
            tc.tile_pool(name="ps_agg", bufs=2, space="PSUM"))
        ps_co = ctx.enter_context(
            tc.tile_pool(name="ps_co", bufs=1, space="PSUM"))

        AGGC = (0, 112, 224, 336, 512, 624)   # aggT chunk cols (bank-safe)

        for t in range(NT):
            # ---------------- phase A for this slot: h, hX ----------------
            xlT = wrk.tile([128, C], bf16, tag="xlT")
            nc.vector.scalar_tensor_tensor(
                xlT[:], xT_all[:, C * t:C * (t + 1)], NEG_IN,
                xT_all[:, C * t:C * (t + 1)], OP.mult, OP.max)
            pre_p = ps_pre.tile([128, 512], f32, tag="pre")
            hp_p = pre_p[:, 480:512]
            nc.tensor.matmul(hp_p, xlT[:], w_in_sb[:], start=True,
                             stop=True, skip_group_check=True)
            h_sb = wrk.tile([128, W], bf16, tag="h")
            nc.scalar.activation(h_sb[:], hp_p, AF.Prelu, bias=0.0,
                                 scale=1.0, alpha=a1_sb[:, 0:1])
            hX = wrk.tile([128, W * KC], bf16, tag="hX")
            nc.vector.tensor_copy(
                hX[:].rearrange("p (w c) -> p w c", c=KC),
                h_sb[:].unsqueeze(-1).broadcast_to([128, W, KC]))

            # ---------------- geometry (dest rows of slot) ----------------
            da = da_t[t % 2]
            dav = da[:].rearrange("p (k d) -> p k d", d=32)
            nbp_t = nbp_all[:, 51 * t:51 * (t + 1)] \
                .rearrange("p (k a) -> p k a", a=3)
            posc_t = posc_all[:, 3 * t:3 * (t + 1)]
            nbo_t = nbo_all[:, 153 * t:153 * (t + 1)] \
                .rearrange("p (k e) -> p k e", e=9)
            oric_t = oric_all[:, 9 * t:9 * (t + 1)]
            mask_t = mask_all[:, K18 * t:K18 * (t + 1)]
            ncl_t = ncl_all[:, t:t + 1]

            D = wrk.tile([128, K17 * 3], bf16, tag="D")
            Dv = D[:].rearrange("p (k a) -> p k a", a=3)
            nc.vector.tensor_sub(Dv, nbp_t,
                                 posc_t.unsqueeze(1).broadcast_to([128, K17, 3]))
            sq = wrk.tile([128, K17 * 3], bf16, tag="sq")
            nc.gpsimd.tensor_mul(sq[:], D[:], D[:])
            d2 = wrk.tile([128, K17], f32, tag="d2")
            nc.vector.tensor_reduce(
                d2[:], sq[:].rearrange("p (k a) -> p k a", a=3),
                axis=AX.X, op=OP.add)
            nc.scalar.activation(dav[:, 0:K17, 6], d2[:], AF.Sqrt,
                                 bias=0.0, scale=1.0 / (R * R))
            dist = wrk.tile([128, K17], f32, tag="dist")
            nc.scalar.activation(dist[:], d2[:], AF.Sqrt, bias=0.0,
                                 scale=1.0)
            nc.vector.tensor_scalar_add(dist[:], dist[:], 1e-9)
            rs = wrk.tile([128, K17], f32, tag="rs")
            nc.vector.reciprocal(rs[:], dist[:])
            dirn = wrk.tile([128, K17 * 3], bf16, tag="dirn")
            nc.vector.scalar_tensor_tensor(
                dirn[:].rearrange("p (k a) -> p k a", a=3), Dv, 1.0,
                rs[:].unsqueeze(-1).broadcast_to([128, K17, 3]),
                OP.mult, OP.mult)
            lm = wrk.tile([128, K17 * 9], bf16, tag="lm")
            nc.vector.tensor_mul(
                lm[:].rearrange("p (k a b) -> p k a b", a=3, b=3),
                dirn[:].rearrange("p (k b) -> p k b", b=3).unsqueeze(2)
                       .broadcast_to([128, K17, 3, 3]),
                oric_t.rearrange("p (a b) -> p a b", b=3).unsqueeze(1)
                      .broadcast_to([128, K17, 3, 3]))
            lo = wrk.tile([128, K17 * 6], f32, tag="lo")
            lov = lo[:].rearrange("p (k d) -> p k d", d=6)
            nc.vector.tensor_reduce(
                lov[:, :, 0:3],
                lm[:].rearrange("p (k a b) -> p k a b", a=3, b=3),
                axis=AX.X, op=OP.add)
            om = wrk.tile([128, K17 * 9], bf16, tag="om")
            nc.vector.tensor_mul(
                om[:].rearrange("p (k a b) -> p k a b", a=3, b=3),
                nbo_t.rearrange("p k (a b) -> p k a b", b=3),
                oric_t.rearrange("p (a b) -> p a b", b=3).unsqueeze(1)
                      .broadcast_to([128, K17, 3, 3]))
            nc.vector.tensor_reduce(
                lov[:, :, 3:6],
                om[:].rearrange("p (k a b) -> p k a b", a=3, b=3),
                axis=AX.X, op=OP.add)
            nc.vector.tensor_copy(dav[:, 0:K17, 0:6], lov)
            nc.vector.tensor_scalar_mul(dav[:, 17, 3:6], lov[:, 8, 3:6],
                                        ncl_t)
            nc.vector.tensor_copy(dav[:, :, 7], mask_t)

            # ------------- daT = transpose(da), 6 chunks of 3 k-slots -----
            daTs = daTs_t[t % 2]
            if STAGE >= 2:
                daT_p = ps_daT.tile([128, 1024], f32, tag="daT")
                for g in range(6):
                    nc.tensor.matmul(daT_p[0:96, 128 * g:128 * (g + 1)],
                                     da[:, 96 * g:96 * (g + 1)], idb[:],
                                     start=True, stop=True,
                                     skip_group_check=True)
                nc.scalar.copy(
                    daTs[0:96].rearrange("p (g c) -> p g c", c=144)
                    [:, :, 8:136],
                    daT_p[0:96, 0:768].rearrange("p (g c) -> p g c", c=128))

            # ---------------- pre_k + prelu -> kernS ----------------------
            kernS = wrk.tile([128, K18 * KC], bf16, tag="kernS")
            if STAGE >= 3:
                for k in range(K18):
                    ksh = _ksh(k)
                    g = k // 3
                    c0 = 144 * g + 16 - ksh
                    # contract over the chunk's full 96 rows; ws_sb rows
                    # outside this k's 8 lanes are zero, so base stays 0
                    nc.tensor.matmul(pre_p[:, KC * k:KC * (k + 1)],
                                     daTs[0:96, c0:c0 + 128],
                                     ws_sb[0:96, KC * k:KC * (k + 1)],
                                     start=True, stop=True,
                                     skip_group_check=True)
                nc.scalar.activation(kernS[:], pre_p[:, 0:K18 * KC], AF.Prelu,
                                     bias=0.0, scale=1.0, alpha=a1_sb[:, 0:1])
            else:
                nc.vector.memset(kernS[:], 0.125)

            # ---------------- products + transpose-accumulate -------------
            # comp slot k=17 shares S_8 (same shift): add kernels, one
            # fewer product
            kernC = wrk.tile([128, KC], bf16, tag="kernC")
            nc.vector.tensor_add(kernC[:], kernS[:, KC * 8:KC * 9],
                                 kernS[:, KC * 17:KC * 18])
            aggTs = wrk.tile([128, 736], bf16, tag="aggTs")
            if STAGE < 1:
                nc.vector.memset(aggTs[:], 0.01)
            aggT_p = ps_agg.tile([128, 1024], f32, tag="aggT")
            for k in range(K17 if STAGE >= 1 else 0):
                tm = tmp.tile([128, W * KC], bf16, tag="tm")
                ksrc = kernC[:, 0:KC] if k == 8 \
                    else kernS[:, KC * k:KC * (k + 1)]
                kv = ksrc.unsqueeze(1).broadcast_to([128, W, KC])
                hv = hX[:].rearrange("p (w c) -> p w c", c=KC)
                tv = tm[:].rearrange("p (w c) -> p w c", c=KC)
                if k in (5, 12):
                    nc.gpsimd.tensor_mul(tv, hv, kv)
                else:
                    nc.vector.scalar_tensor_tensor(tv, hv, 1.0, kv,
                                                   OP.mult, OP.mult)
                for b in range(6):
                    nc.tensor.matmul(
                        aggT_p[:, AGGC[b]:AGGC[b] + TS],
                        tm[:, 128 * b:128 * (b + 1)],
                        s_sb[:, TS * k:TS * (k + 1)],
                        start=(k == 0 and b in (0, 4)),
                        stop=(k == K17 - 1 and b in (3, 5)),
                        skip_group_check=True)
            if STAGE >= 1:
                nc.scalar.copy(aggTs[:], aggT_p[:, 0:736])

            # ---------------- conv = lrelu(Wk @ aggT); out ----------------
            co_p = ps_co.tile([128, 512], f32, tag="co")
            for b in range(6):
                nc.tensor.matmul(co_p[0:W, 0:TS], wk_sb[:, W * b:W * (b + 1)],
                                 aggTs[:, AGGC[b]:AGGC[b] + TS],
                                 start=(b == 0), stop=(b == 5),
                                 skip_group_check=True)
            convL = tmp.tile([W, TS], bf16, tag="convL")
            nc.scalar.activation(convL[:], co_p[0:W, 0:TS], AF.Prelu,
                                 bias=0.0, scale=1.0, alpha=a1_sb[0:W, 0:1])
            nc.tensor.matmul(co_p[0:TS, 128:256], convL[:], w_out_sb[:],
                             start=True, stop=False, skip_group_check=True)
            nc.tensor.matmul(co_p[0:TS, 128:256], idb[0:TS, 0:TS],
                             xc_all[0:TS, C * t:C * (t + 1)],
                             start=False, stop=True, skip_group_check=True)
            out_sb = wrk.tile([TS, C], f32, tag="out_sb")
            nc.scalar.copy(out_sb[:], co_p[0:TS, 128:256])
            cnt = min(TS, NPC - TS * t)
            nc.sync.dma_start(y[TS * t:TS * t + cnt, :], out_sb[0:cnt, :])

    nc.compile()
    return nc


def _expected_src_dst():
    i = np.arange(N)
    offs = np.arange(-WIN, WIN + 1)
    j = i[:, None] + offs[None, :]
    valid = ((j // L) == (i[:, None] // L)) & (j >= 0) & (j < N)
    j = np.where(valid, j, i[:, None])
    dst = np.repeat(i, offs.size).astype(np.int32)
    src = j.reshape(-1).astype(np.int32)
    return src, dst


def _host_inputs(x, pos, ori, W_in, Ws0, bs0, Wk, W_out):
    xf = np.ascontiguousarray(x.reshape(N, C), np.float32)
    pos = np.asarray(pos, np.float32)
    ori = np.asarray(ori, np.float32)

    def bf(a):
        return np.asarray(a, BF)

    # Wk rows permuted to (w, c) order, in 6 chunks of 128 rows
    Wk_wc = np.empty_like(Wk)
    for c in range(KC):
        for w in range(W):
            Wk_wc[w * KC + c] = Wk[c * W + w]
    wk_p = np.zeros((128, 6 * W), np.float32)
    for b in range(6):
        wk_p[:, W * b:W * (b + 1)] = Wk_wc[128 * b:128 * (b + 1), :]

    # ws_sb: col-block k holds WS_k rows at partitions 32*(k%4)+d
    ws_sb = np.zeros((128, K18 * KC), np.float32)
    for k in range(K17):
        s = _sidx(k)
        r0 = 32 * (k % 3)
        ws_sb[r0:r0 + 7, KC * k:KC * (k + 1)] = Ws0[s]
        ws_sb[r0 + 7, KC * k:KC * (k + 1)] = bs0[s]
    r0 = 32 * (17 % 3)
    ws_sb[r0 + 3:r0 + 6, KC * 17:KC * 18] = Ws0[5][3:6]
    ws_sb[r0 + 7, KC * 17:KC * 18] = bs0[5]

    # shifted identities S_k[m, j] = 1 iff j = m - ksh
    s_mats = np.zeros((128, K18 * TS), np.float32)
    for k in range(K18):
        ksh = _ksh(k)
        for j in range(TS):
            m = j + ksh
            if 0 <= m < 128:
                s_mats[m, TS * k + j] = 1.0

    common = dict(
        w_in=bf(W_in),
        ws_sb=bf(ws_sb),
        s_mats=bf(s_mats),
        idb=bf(np.eye(128, dtype=np.float32)),
        wk_p=bf(wk_p),
        w_out=bf(W_out),
        alph1=np.full((128, 1), NEG_IN, np.float32),
    )

    offs = np.arange(-WIN, WIN + 1)
    in_maps = []
    for ci in range(NCORES):
        s0 = ci * NPC
        g = s0 - WIN + np.arange(HR)
        ok = (g >= 0) & (g < N)
        gi = np.clip(g, 0, N - 1)
        x_pad = np.where(ok[:, None], xf[gi], 0.0).astype(np.float32)
        pos_pad = np.where(ok[:, None], pos[gi], 0.0).astype(np.float32)
        ori_pad = np.where(ok[:, None], ori[gi], 0.0).astype(np.float32)

        jj, pp = np.meshgrid(np.arange(NT), np.arange(128), indexing="ij")
        rows = (TS * jj + pp)                      # [NT,128] pad-row index
        xT_slot = x_pad[rows].transpose(2, 0, 1).reshape(C, NT * 128)
        rc = WIN + TS * jj + pp
        okc = rc < HR
        xc_slot = np.where(okc[:, :, None], x_pad[np.minimum(rc, HR - 1)], 0.0)
        xc_slot = xc_slot.transpose(1, 0, 2).reshape(128, NT * C) \
            .astype(np.float32)

        gdest = s0 - WIN + rows                    # [NT,128] global dest node
        nb_g = gdest[:, :, None] + offs[None, None, :]
        valid = ((nb_g // L) == (gdest[:, :, None] // L)) \
            & (nb_g >= 0) & (nb_g < N)
        nb_gi = np.where(valid, np.clip(nb_g, 0, N - 1),
                         np.clip(gdest[:, :, None], 0, N - 1))
        nbp_a = np.where(valid[..., None], pos[nb_gi],
                         pos_pad[rows][:, :, None, :])      # [NT,128,17,3]
        nbo_a = np.where(valid[..., None], ori[nb_gi], 0.0)  # [NT,128,17,9]
        mask = valid.astype(np.float32)                      # [NT,128,17]
        ncl = (K17 - mask.sum(-1)).astype(np.float32)        # [NT,128]
        maskb = np.concatenate([mask, ncl[:, :, None]], axis=2)

        in_maps.append(dict(
            xT_slot=np.ascontiguousarray(xT_slot, np.float32),
            xc_slot=bf(xc_slot),
            nbp=nbp_a.transpose(1, 0, 2, 3).reshape(128, NT * K17 * 3)
                .astype(np.float32),
            posc=pos_pad[rows].transpose(1, 0, 2).reshape(128, NT * 3)
                .astype(np.float32),
            nbo=bf(nbo_a.transpose(1, 0, 2, 3).reshape(128, NT * K17 * 9)),
            oric=bf(ori_pad[rows].transpose(1, 0, 2).reshape(128, NT * 9)),
            maskb=bf(maskb.transpose(1, 0, 2).reshape(128, NT * K18)),
            nclf=np.ascontiguousarray(ncl.T, np.float32),
            **common))
    return in_maps


def kernel(x, pos, seq, ori, W_in, Ws0, bs0, Wk, W_out, src, dst):
    exp_src, exp_dst = _expected_src_dst()
    assert np.array_equal(np.asarray(src), exp_src), "unexpected src graph"
    assert np.array_equal(np.asarray(dst), exp_dst), "unexpected dst graph"

    from concourse.bass_utils import run_bass_kernel_spmd

    if "nc" not in _PROG:
        _PROG["nc"] = _build_program()
    nc = _PROG["nc"]

    in_maps = _host_inputs(np.asarray(x), np.asarray(pos), np.asarray(ori),
                           np.asarray(W_in), np.asarray(Ws0), np.asarray(bs0),
                           np.asarray(Wk), np.asarray(W_out))
    res = run_bass_kernel_spmd(nc, in_maps, list(range(NCORES)))
    out = np.concatenate([res.results[i]["y"] for i in range(NCORES)], axis=0)
    return out.reshape(B, L, C).astype(np.float32)
